# revision 9
# baseline (speedup 1.0000x reference)
"""Trainium2 Bass kernel for gnn_message_passing (nn_BuildK_25005299597348).

Reference computation:
    UU = input1.reshape(32, N).T              # [N, 32] pixel features
    nbr = UU[input2]                          # [J, 48, 32] neighbor gather
    msd = mean((UU[:J, None, :] - nbr)**2, -1)
    W = softmax(-sqrt(msd + 1e-9), axis=1)    # [J, 48]

Strategy (8 NeuronCores, data-parallel over query rows):
  - The TRN2 indirect-DMA primitive consumes ONE offset per partition per
    instruction (one contiguous descriptor per partition), so an on-device
    row gather costs J*K/128 = 7056 Pool instructions per core - that per-
    instruction SWDGE overhead, not bandwidth, is the old bottleneck.
  - Instead the host lays each query's K neighbor rows out contiguously
    (pure data movement; fp16 halves the bytes) so the device streams them
    with large per-partition descriptors at full DMA bandwidth.
  - Streams are FEATURE-MAJOR (f, k, t): every halving step of the mean-
    reduce tree is then a contiguous fp16 add (DVE 2x perf mode, measured
    0.53 ns/elem vs 1.06 for grouped tensor_reduce), and the query-
    broadcast subtract keeps a packed innermost dim. ACT squares in place;
    gpsimd takes a slice of tree step 1; sqrt + fused softmax over the 48
    neighbors finish on ACT/DVE. The host un-permutes the (k, t)-ordered
    output rows (pure data movement again).
"""

import sys

for _p in ("/opt/trn_rl_repo", "/root/.axon_site/_ro/trn_rl_repo"):
    if _p not in sys.path:
        sys.path.append(_p)

import numpy as np

import concourse.bass as bass
import concourse.bacc as bacc
import concourse.mybir as mybir
import concourse.tile as tile

F32 = mybir.dt.float32
F16 = mybir.dt.float16

N = 147456          # pixels (384*384)
A = 32              # features
K = 48              # neighbors
NCORES = 8
JC = N // NCORES    # queries per core (18432)
P = 128             # partitions
T = 8               # rows per partition per supertile
EPS = 1e-9
POOL_ELEMS = 3072   # contiguous slice of tree step 1 handled by gpsimd


def build_kernel(a=A, k=K, jc=JC):
    """Build the SPMD Bass program. Returns nc."""
    sup = jc // (P * T)             # supertiles per core (18)
    kt = k * T                      # (k, t) slots per partition (384)
    e = kt * a                      # elems per partition per supertile (12288)

    nc = bacc.Bacc(None, target_bir_lowering=False)
    eps_t = nc.alloc_sbuf_tensor("const-eps", [P, 1], F32)
    nc.gpsimd.memset(eps_t.ap(), EPS)
    nc.const_aps.aps[(F32, EPS)] = eps_t.ap()
    nc.all_engine_barrier()

    # feature-major neighbor stream: nbr[s*P+p, f*kt + kk*T + t]
    nbr = nc.declare_dram_parameter("nbr", [sup * P, e], F16, isOutput=False)
    # transposed query features: qf[s*P+p, f*T + t]
    qf = nc.declare_dram_parameter("qf", [sup * P, a * T], F16, isOutput=False)
    # (k, t)-ordered output rows, host un-permutes
    out = nc.declare_dram_parameter("out", [sup * P, kt], F32, isOutput=True)

    nbr_v = nbr[:].rearrange("(s p) e -> s p e", p=P)
    qf_v = qf[:].rearrange("(s p) e -> s p e", p=P)
    out_v = out[:].rearrange("(s p) e -> s p e", p=P)

    with tile.TileContext(nc) as tc:
        with (
            tc.tile_pool(name="pg", bufs=2) as pg,
            tc.tile_pool(name="pdf", bufs=2) as pdf,
            tc.tile_pool(name="ph", bufs=2) as ph,
            tc.tile_pool(name="pq", bufs=3) as pq,
            tc.tile_pool(name="psm", bufs=2) as psm,
            tc.tile_pool(name="pty", bufs=2) as pty,
        ):
            for s in range(sup):
                q = pq.tile([P, a * T], F16)
                nc.sync.dma_start(out=q[:], in_=qf_v[s])
                g = pg.tile([P, e], F16)
                nc.sync.dma_start(out=g[:], in_=nbr_v[s])
                diff = pdf.tile([P, e], F16)
                nc.vector.tensor_tensor(
                    out=diff[:].rearrange("p (f k t) -> p f k t", f=a, k=k),
                    in0=g[:].rearrange("p (f k t) -> p f k t", f=a, k=k),
                    in1=q[:].rearrange("p (f o t) -> p f o t", o=1, t=T).to_broadcast(
                        [P, a, k, T]
                    ),
                    op=mybir.AluOpType.subtract,
                )
                # square in place on ACT
                nc.scalar.square(out=diff[:], in_=diff[:])
                # contiguous halving tree over the feature-major dim:
                # 32 -> 16 -> 8 -> 4 -> 2 -> 1 slots of kt values each
                h1 = ph.tile([P, e // 2], F16)
                pe = POOL_ELEMS
                nc.gpsimd.tensor_tensor(
                    out=h1[:, 0:pe], in0=diff[:, 0:pe],
                    in1=diff[:, e // 2:e // 2 + pe], op=mybir.AluOpType.add,
                )
                nc.vector.tensor_tensor(
                    out=h1[:, pe:e // 2], in0=diff[:, pe:e // 2],
                    in1=diff[:, e // 2 + pe:e], op=mybir.AluOpType.add,
                )
                h2 = ph.tile([P, e // 4], F16)
                nc.vector.tensor_tensor(
                    out=h2[:], in0=h1[:, 0:e // 4], in1=h1[:, e // 4:e // 2],
                    op=mybir.AluOpType.add,
                )
                h3 = ph.tile([P, e // 8], F16)
                nc.vector.tensor_tensor(
                    out=h3[:], in0=h2[:, 0:e // 8], in1=h2[:, e // 8:e // 4],
                    op=mybir.AluOpType.add,
                )
                h4 = ph.tile([P, e // 16], F16)
                nc.vector.tensor_tensor(
                    out=h4[:], in0=h3[:, 0:e // 16], in1=h3[:, e // 16:e // 8],
                    op=mybir.AluOpType.add,
                )
                ss = psm.tile([P, kt], F16)
                nc.vector.tensor_tensor(
                    out=ss[:], in0=h4[:, 0:kt], in1=h4[:, kt:2 * kt],
                    op=mybir.AluOpType.add,
                )
                # sd = sqrt(ss/a + eps); D = -sd
                sd = psm.tile([P, kt], F16)
                nc.scalar.activation(
                    out=sd[:], in_=ss[:], func=mybir.ActivationFunctionType.Sqrt,
                    bias=EPS, scale=1.0 / a,
                )
                mn = pty.tile([P, T], F16)
                nc.vector.tensor_reduce(
                    out=mn[:],
                    in_=sd[:].rearrange("p (k t) -> p t k", t=T),
                    axis=mybir.AxisListType.X,
                    op=mybir.AluOpType.min,
                )
                sm = psm.tile([P, kt], F16)
                nc.vector.tensor_tensor(
                    out=sm[:].rearrange("p (k t) -> p k t", t=T),
                    in0=sd[:].rearrange("p (k t) -> p k t", t=T),
                    in1=mn[:].rearrange("p (o t) -> p o t", o=1).to_broadcast(
                        [P, k, T]
                    ),
                    op=mybir.AluOpType.subtract,
                )
                ex = psm.tile([P, kt], F16)
                nc.scalar.activation(
                    out=ex[:], in_=sm[:], func=mybir.ActivationFunctionType.Exp,
                    scale=-1.0,
                )
                se = pty.tile([P, T], F32)
                nc.vector.tensor_reduce(
                    out=se[:],
                    in_=ex[:].rearrange("p (k t) -> p t k", t=T),
                    axis=mybir.AxisListType.X,
                    op=mybir.AluOpType.add,
                )
                rc = pty.tile([P, T], F32)
                nc.vector.reciprocal(out=rc[:], in_=se[:])
                wt = psm.tile([P, kt], F32)
                nc.vector.tensor_tensor(
                    out=wt[:].rearrange("p (k t) -> p k t", t=T),
                    in0=ex[:].rearrange("p (k t) -> p k t", t=T),
                    in1=rc[:].rearrange("p (o t) -> p o t", o=1).to_broadcast(
                        [P, k, T]
                    ),
                    op=mybir.AluOpType.mult,
                )
                nc.gpsimd.dma_start(out=out_v[s], in_=wt[:])
    return nc


_compiled = {}


def _run(input1, input2, trace=False, **trace_kwargs):
    from concourse.bass_utils import run_bass_kernel_spmd

    sup = JC // (P * T)
    uu16 = np.ascontiguousarray(
        np.asarray(input1, dtype=np.float32).reshape(A, N).T.astype(np.float16)
    )
    idx = np.asarray(input2).astype(np.int64).ravel()
    # host layout transform to feature-major (s, p, f, k, t)
    nbr_g = uu16[idx].reshape(NCORES * sup, P, T, K, A)     # (S, p, t, k, f)
    nbr_fm = np.ascontiguousarray(nbr_g.transpose(0, 1, 4, 3, 2)).reshape(
        NCORES * sup * P, K * T * A
    )
    qf_t = np.ascontiguousarray(
        uu16.reshape(NCORES * sup, P, T, A).transpose(0, 1, 3, 2)
    ).reshape(NCORES * sup * P, A * T)

    if "nc" not in _compiled:
        nc = build_kernel()
        nc.finalize()
        _compiled["nc"] = nc
    nc = _compiled["nc"]

    spp = sup * P
    in_maps = [
        {
            "nbr": nbr_fm[c * spp:(c + 1) * spp],
            "qf": qf_t[c * spp:(c + 1) * spp],
        }
        for c in range(NCORES)
    ]
    res = run_bass_kernel_spmd(
        nc, in_maps, list(range(NCORES)), trace=trace, **trace_kwargs
    )
    # un-permute (s, p, k, t) -> row-major [J, K]
    out = np.concatenate(
        [
            res.results[c]["out"]
            .reshape(sup, P, K, T)
            .transpose(0, 1, 3, 2)
            .reshape(JC, K)
            for c in range(NCORES)
        ],
        axis=0,
    )
    return out, res


def kernel(input1: np.ndarray, input2: np.ndarray) -> np.ndarray:
    out, _ = _run(input1, input2)
    return out


# revision 10
# speedup vs baseline: 1.0559x; 1.0559x over previous
"""Trainium2 Bass kernel for gnn_message_passing (nn_BuildK_25005299597348).

Reference computation:
    UU = input1.reshape(32, N).T              # [N, 32] pixel features
    nbr = UU[input2]                          # [J, 48, 32] neighbor gather
    msd = mean((UU[:J, None, :] - nbr)**2, -1)
    W = softmax(-sqrt(msd + 1e-9), axis=1)    # [J, 48]

Strategy (8 NeuronCores, data-parallel over query rows):
  - The TRN2 indirect-DMA primitive consumes ONE offset per partition per
    instruction (one contiguous descriptor per partition), so an on-device
    row gather costs J*K/128 = 7056 Pool instructions per core - that per-
    instruction SWDGE overhead, not bandwidth, is the old bottleneck.
  - Instead the host lays each query's K neighbor rows out contiguously
    (pure data movement; fp16 halves the bytes) so the device streams them
    with large per-partition descriptors at full DMA bandwidth.
  - Streams are FEATURE-MAJOR (f, k, t): every halving step of the mean-
    reduce tree is then a contiguous fp16 add (DVE 2x perf mode, measured
    0.53 ns/elem vs 1.06 for grouped tensor_reduce), and the query-
    broadcast subtract keeps a packed innermost dim. ACT squares in place;
    gpsimd takes a slice of tree step 1; sqrt + fused softmax over the 48
    neighbors finish on ACT/DVE. The host un-permutes the (k, t)-ordered
    output rows (pure data movement again).
"""

import sys

for _p in ("/opt/trn_rl_repo", "/root/.axon_site/_ro/trn_rl_repo"):
    if _p not in sys.path:
        sys.path.append(_p)

import numpy as np

import concourse.bass as bass
import concourse.bacc as bacc
import concourse.mybir as mybir
import concourse.tile as tile

F32 = mybir.dt.float32
F16 = mybir.dt.float16

N = 147456          # pixels (384*384)
A = 32              # features
K = 48              # neighbors
NCORES = 8
JC = N // NCORES    # queries per core (18432)
P = 128             # partitions
T = 8               # rows per partition per supertile
EPS = 1e-9
POOL_ELEMS = 2048   # contiguous slice of tree step 1 handled by gpsimd


def build_kernel(a=A, k=K, jc=JC):
    """Build the SPMD Bass program. Returns nc."""
    sup = jc // (P * T)             # supertiles per core (18)
    kt = k * T                      # (k, t) slots per partition (384)
    e = kt * a                      # elems per partition per supertile (12288)

    nc = bacc.Bacc(None, target_bir_lowering=False)
    eps_t = nc.alloc_sbuf_tensor("const-eps", [P, 1], F32)
    nc.gpsimd.memset(eps_t.ap(), EPS)
    nc.const_aps.aps[(F32, EPS)] = eps_t.ap()
    nc.all_engine_barrier()

    # feature-major neighbor stream: nbr[s*P+p, f*kt + kk*T + t]
    nbr = nc.declare_dram_parameter("nbr", [sup * P, e], F16, isOutput=False)
    # transposed query features: qf[s*P+p, f*T + t]
    qf = nc.declare_dram_parameter("qf", [sup * P, a * T], F16, isOutput=False)
    # (k, t)-ordered output rows, host un-permutes
    out = nc.declare_dram_parameter("out", [sup * P, kt], F32, isOutput=True)

    nbr_v = nbr[:].rearrange("(s p) e -> s p e", p=P)
    qf_v = qf[:].rearrange("(s p) e -> s p e", p=P)
    out_v = out[:].rearrange("(s p) e -> s p e", p=P)

    with tile.TileContext(nc) as tc:
        with (
            tc.tile_pool(name="pg", bufs=2) as pg,
            tc.tile_pool(name="pdf", bufs=2) as pdf,
            tc.tile_pool(name="ph", bufs=2) as ph,
            tc.tile_pool(name="pq", bufs=3) as pq,
            tc.tile_pool(name="psm", bufs=2) as psm,
            tc.tile_pool(name="pty", bufs=2) as pty,
        ):
            for s in range(sup):
                q = pq.tile([P, a * T], F16)
                nc.sync.dma_start(out=q[:], in_=qf_v[s])
                g = pg.tile([P, e], F16)
                nc.sync.dma_start(out=g[:], in_=nbr_v[s])
                diff = pdf.tile([P, e], F16)
                nc.vector.tensor_tensor(
                    out=diff[:].rearrange("p (f k t) -> p f k t", f=a, k=k),
                    in0=g[:].rearrange("p (f k t) -> p f k t", f=a, k=k),
                    in1=q[:].rearrange("p (f o t) -> p f o t", o=1, t=T).to_broadcast(
                        [P, a, k, T]
                    ),
                    op=mybir.AluOpType.subtract,
                )
                # square in place on ACT
                nc.scalar.square(out=diff[:], in_=diff[:])
                # contiguous halving tree over the feature-major dim:
                # 32 -> 16 -> 8 -> 4 -> 2 -> 1 slots of kt values each
                h1 = ph.tile([P, e // 2], F16)
                pe = POOL_ELEMS
                nc.gpsimd.tensor_tensor(
                    out=h1[:, 0:pe], in0=diff[:, 0:pe],
                    in1=diff[:, e // 2:e // 2 + pe], op=mybir.AluOpType.add,
                )
                nc.vector.tensor_tensor(
                    out=h1[:, pe:e // 2], in0=diff[:, pe:e // 2],
                    in1=diff[:, e // 2 + pe:e], op=mybir.AluOpType.add,
                )
                h2 = ph.tile([P, e // 4], F16)
                nc.vector.tensor_tensor(
                    out=h2[:], in0=h1[:, 0:e // 4], in1=h1[:, e // 4:e // 2],
                    op=mybir.AluOpType.add,
                )
                h3 = ph.tile([P, e // 8], F16)
                nc.vector.tensor_tensor(
                    out=h3[:], in0=h2[:, 0:e // 8], in1=h2[:, e // 8:e // 4],
                    op=mybir.AluOpType.add,
                )
                h4 = ph.tile([P, e // 16], F16)
                nc.vector.tensor_tensor(
                    out=h4[:], in0=h3[:, 0:e // 16], in1=h3[:, e // 16:e // 8],
                    op=mybir.AluOpType.add,
                )
                ss = psm.tile([P, kt], F16)
                nc.vector.tensor_tensor(
                    out=ss[:], in0=h4[:, 0:kt], in1=h4[:, kt:2 * kt],
                    op=mybir.AluOpType.add,
                )
                # sd = sqrt(ss/a + eps); D = -sd
                sd = psm.tile([P, kt], F16)
                nc.scalar.activation(
                    out=sd[:], in_=ss[:], func=mybir.ActivationFunctionType.Sqrt,
                    bias=EPS, scale=1.0 / a,
                )
                mn = pty.tile([P, T], F16)
                nc.vector.tensor_reduce(
                    out=mn[:],
                    in_=sd[:].rearrange("p (k t) -> p t k", t=T),
                    axis=mybir.AxisListType.X,
                    op=mybir.AluOpType.min,
                )
                sm = psm.tile([P, kt], F16)
                nc.vector.tensor_tensor(
                    out=sm[:].rearrange("p (k t) -> p k t", t=T),
                    in0=sd[:].rearrange("p (k t) -> p k t", t=T),
                    in1=mn[:].rearrange("p (o t) -> p o t", o=1).to_broadcast(
                        [P, k, T]
                    ),
                    op=mybir.AluOpType.subtract,
                )
                ex = psm.tile([P, kt], F16)
                nc.scalar.activation(
                    out=ex[:], in_=sm[:], func=mybir.ActivationFunctionType.Exp,
                    scale=-1.0,
                )
                se = pty.tile([P, T], F32)
                nc.vector.tensor_reduce(
                    out=se[:],
                    in_=ex[:].rearrange("p (k t) -> p t k", t=T),
                    axis=mybir.AxisListType.X,
                    op=mybir.AluOpType.add,
                )
                rc = pty.tile([P, T], F32)
                nc.vector.reciprocal(out=rc[:], in_=se[:])
                wt = psm.tile([P, kt], F32)
                nc.vector.tensor_tensor(
                    out=wt[:].rearrange("p (k t) -> p k t", t=T),
                    in0=ex[:].rearrange("p (k t) -> p k t", t=T),
                    in1=rc[:].rearrange("p (o t) -> p o t", o=1).to_broadcast(
                        [P, k, T]
                    ),
                    op=mybir.AluOpType.mult,
                )
                nc.gpsimd.dma_start(out=out_v[s], in_=wt[:])
    return nc


_compiled = {}


def _run(input1, input2, trace=False, **trace_kwargs):
    from concourse.bass_utils import run_bass_kernel_spmd

    sup = JC // (P * T)
    uu16 = np.ascontiguousarray(
        np.asarray(input1, dtype=np.float32).reshape(A, N).T.astype(np.float16)
    )
    idx = np.asarray(input2).astype(np.int64).ravel()
    # host layout transform to feature-major (s, p, f, k, t)
    nbr_g = uu16[idx].reshape(NCORES * sup, P, T, K, A)     # (S, p, t, k, f)
    nbr_fm = np.ascontiguousarray(nbr_g.transpose(0, 1, 4, 3, 2)).reshape(
        NCORES * sup * P, K * T * A
    )
    qf_t = np.ascontiguousarray(
        uu16.reshape(NCORES * sup, P, T, A).transpose(0, 1, 3, 2)
    ).reshape(NCORES * sup * P, A * T)

    if "nc" not in _compiled:
        nc = build_kernel()
        nc.finalize()
        _compiled["nc"] = nc
    nc = _compiled["nc"]

    spp = sup * P
    in_maps = [
        {
            "nbr": nbr_fm[c * spp:(c + 1) * spp],
            "qf": qf_t[c * spp:(c + 1) * spp],
        }
        for c in range(NCORES)
    ]
    res = run_bass_kernel_spmd(
        nc, in_maps, list(range(NCORES)), trace=trace, **trace_kwargs
    )
    # un-permute (s, p, k, t) -> row-major [J, K]
    out = np.concatenate(
        [
            res.results[c]["out"]
            .reshape(sup, P, K, T)
            .transpose(0, 1, 3, 2)
            .reshape(JC, K)
            for c in range(NCORES)
        ],
        axis=0,
    )
    return out, res


def kernel(input1: np.ndarray, input2: np.ndarray) -> np.ndarray:
    out, _ = _run(input1, input2)
    return out


# revision 11
# speedup vs baseline: 1.0950x; 1.0370x over previous
"""Trainium2 Bass kernel for gnn_message_passing (nn_BuildK_25005299597348).

Reference computation:
    UU = input1.reshape(32, N).T              # [N, 32] pixel features
    nbr = UU[input2]                          # [J, 48, 32] neighbor gather
    msd = mean((UU[:J, None, :] - nbr)**2, -1)
    W = softmax(-sqrt(msd + 1e-9), axis=1)    # [J, 48]

Strategy (8 NeuronCores, data-parallel over query rows):
  - The TRN2 indirect-DMA primitive consumes ONE offset per partition per
    instruction (one contiguous descriptor per partition), so an on-device
    row gather costs J*K/128 = 7056 Pool instructions per core - that per-
    instruction SWDGE overhead, not bandwidth, is the old bottleneck.
  - Instead the host lays each query's K neighbor rows out contiguously
    (pure data movement; fp16 halves the bytes) so the device streams them
    with large per-partition descriptors at full DMA bandwidth.
  - Streams are FEATURE-MAJOR (f, k, t): every halving step of the mean-
    reduce tree is then a contiguous fp16 add (DVE 2x perf mode, measured
    0.53 ns/elem vs 1.06 for grouped tensor_reduce), and the query-
    broadcast subtract keeps a packed innermost dim. ACT squares in place;
    gpsimd takes a slice of tree step 1; sqrt + fused softmax over the 48
    neighbors finish on ACT/DVE. The host un-permutes the (k, t)-ordered
    output rows (pure data movement again).
"""

import sys

for _p in ("/opt/trn_rl_repo", "/root/.axon_site/_ro/trn_rl_repo"):
    if _p not in sys.path:
        sys.path.append(_p)

import numpy as np

import concourse.bass as bass
import concourse.bacc as bacc
import concourse.mybir as mybir
import concourse.tile as tile

F32 = mybir.dt.float32
F16 = mybir.dt.float16

N = 147456          # pixels (384*384)
A = 32              # features
K = 48              # neighbors
NCORES = 8
JC = N // NCORES    # queries per core (18432)
P = 128             # partitions
T = 8               # rows per partition per supertile
EPS = 1e-9
POOL_ELEMS = 2048   # contiguous slice of tree step 1 handled by gpsimd


def build_kernel(a=A, k=K, jc=JC):
    """Build the SPMD Bass program. Returns nc."""
    sup = jc // (P * T)             # supertiles per core (18)
    kt = k * T                      # (k, t) slots per partition (384)
    e = kt * a                      # elems per partition per supertile (12288)

    nc = bacc.Bacc(None, target_bir_lowering=False)
    eps_t = nc.alloc_sbuf_tensor("const-eps", [P, 1], F32)
    nc.gpsimd.memset(eps_t.ap(), EPS)
    nc.const_aps.aps[(F32, EPS)] = eps_t.ap()
    nc.all_engine_barrier()

    # feature-major neighbor stream: nbr[s*P+p, f*kt + kk*T + t]
    nbr = nc.declare_dram_parameter("nbr", [sup * P, e], F16, isOutput=False)
    # transposed query features: qf[s*P+p, f*T + t]
    qf = nc.declare_dram_parameter("qf", [sup * P, a * T], F16, isOutput=False)
    # (k, t)-ordered output rows, host un-permutes
    out = nc.declare_dram_parameter("out", [sup * P, kt], F32, isOutput=True)

    nbr_v = nbr[:].rearrange("(s p) e -> s p e", p=P)
    qf_v = qf[:].rearrange("(s p) e -> s p e", p=P)
    out_v = out[:].rearrange("(s p) e -> s p e", p=P)

    with tile.TileContext(nc) as tc:
        with (
            tc.tile_pool(name="pg", bufs=2) as pg,
            tc.tile_pool(name="pdf", bufs=3) as pdf,
            tc.tile_pool(name="ph", bufs=2) as ph,
            tc.tile_pool(name="pq", bufs=3) as pq,
            tc.tile_pool(name="psm", bufs=2) as psm,
            tc.tile_pool(name="pty", bufs=2) as pty,
        ):
            for s in range(sup):
                q = pq.tile([P, a * T], F16)
                nc.sync.dma_start(out=q[:], in_=qf_v[s])
                g = pg.tile([P, e], F16)
                nc.sync.dma_start(out=g[:], in_=nbr_v[s])
                diff = pdf.tile([P, e], F16)
                nc.vector.tensor_tensor(
                    out=diff[:].rearrange("p (f k t) -> p f k t", f=a, k=k),
                    in0=g[:].rearrange("p (f k t) -> p f k t", f=a, k=k),
                    in1=q[:].rearrange("p (f o t) -> p f o t", o=1, t=T).to_broadcast(
                        [P, a, k, T]
                    ),
                    op=mybir.AluOpType.subtract,
                )
                # square in place on ACT
                nc.scalar.square(out=diff[:], in_=diff[:])
                # contiguous halving tree over the feature-major dim:
                # 32 -> 16 -> 8 -> 4 -> 2 -> 1 slots of kt values each
                h1 = ph.tile([P, e // 2], F16)
                pe = POOL_ELEMS
                nc.gpsimd.tensor_tensor(
                    out=h1[:, 0:pe], in0=diff[:, 0:pe],
                    in1=diff[:, e // 2:e // 2 + pe], op=mybir.AluOpType.add,
                )
                nc.vector.tensor_tensor(
                    out=h1[:, pe:e // 2], in0=diff[:, pe:e // 2],
                    in1=diff[:, e // 2 + pe:e], op=mybir.AluOpType.add,
                )
                h2 = ph.tile([P, e // 4], F16)
                nc.vector.tensor_tensor(
                    out=h2[:], in0=h1[:, 0:e // 4], in1=h1[:, e // 4:e // 2],
                    op=mybir.AluOpType.add,
                )
                h3 = ph.tile([P, e // 8], F16)
                nc.vector.tensor_tensor(
                    out=h3[:], in0=h2[:, 0:e // 8], in1=h2[:, e // 8:e // 4],
                    op=mybir.AluOpType.add,
                )
                h4 = ph.tile([P, e // 16], F16)
                nc.vector.tensor_tensor(
                    out=h4[:], in0=h3[:, 0:e // 16], in1=h3[:, e // 16:e // 8],
                    op=mybir.AluOpType.add,
                )
                ss = psm.tile([P, kt], F16)
                nc.vector.tensor_tensor(
                    out=ss[:], in0=h4[:, 0:kt], in1=h4[:, kt:2 * kt],
                    op=mybir.AluOpType.add,
                )
                # sd = sqrt(ss/a + eps); D = -sd
                sd = psm.tile([P, kt], F16)
                nc.scalar.activation(
                    out=sd[:], in_=ss[:], func=mybir.ActivationFunctionType.Sqrt,
                    bias=EPS, scale=1.0 / a,
                )
                # exp(-sd) directly: sd is bounded in [0, ~8], so the usual
                # max-subtraction stabilization is unnecessary (exp stays in
                # [3e-4, 1], well inside fp16 range)
                ex = psm.tile([P, kt], F16)
                nc.scalar.activation(
                    out=ex[:], in_=sd[:], func=mybir.ActivationFunctionType.Exp,
                    scale=-1.0,
                )
                se = pty.tile([P, T], F32)
                nc.vector.tensor_reduce(
                    out=se[:],
                    in_=ex[:].rearrange("p (k t) -> p t k", t=T),
                    axis=mybir.AxisListType.X,
                    op=mybir.AluOpType.add,
                )
                rc = pty.tile([P, T], F32)
                nc.vector.reciprocal(out=rc[:], in_=se[:])
                wt = psm.tile([P, kt], F32)
                nc.vector.tensor_tensor(
                    out=wt[:].rearrange("p (k t) -> p k t", t=T),
                    in0=ex[:].rearrange("p (k t) -> p k t", t=T),
                    in1=rc[:].rearrange("p (o t) -> p o t", o=1).to_broadcast(
                        [P, k, T]
                    ),
                    op=mybir.AluOpType.mult,
                )
                nc.gpsimd.dma_start(out=out_v[s], in_=wt[:])
    return nc


_compiled = {}


def _run(input1, input2, trace=False, **trace_kwargs):
    from concourse.bass_utils import run_bass_kernel_spmd

    sup = JC // (P * T)
    uu16 = np.ascontiguousarray(
        np.asarray(input1, dtype=np.float32).reshape(A, N).T.astype(np.float16)
    )
    idx = np.asarray(input2).astype(np.int64).ravel()
    # host layout transform to feature-major (s, p, f, k, t)
    nbr_g = uu16[idx].reshape(NCORES * sup, P, T, K, A)     # (S, p, t, k, f)
    nbr_fm = np.ascontiguousarray(nbr_g.transpose(0, 1, 4, 3, 2)).reshape(
        NCORES * sup * P, K * T * A
    )
    qf_t = np.ascontiguousarray(
        uu16.reshape(NCORES * sup, P, T, A).transpose(0, 1, 3, 2)
    ).reshape(NCORES * sup * P, A * T)

    if "nc" not in _compiled:
        nc = build_kernel()
        nc.finalize()
        _compiled["nc"] = nc
    nc = _compiled["nc"]

    spp = sup * P
    in_maps = [
        {
            "nbr": nbr_fm[c * spp:(c + 1) * spp],
            "qf": qf_t[c * spp:(c + 1) * spp],
        }
        for c in range(NCORES)
    ]
    res = run_bass_kernel_spmd(
        nc, in_maps, list(range(NCORES)), trace=trace, **trace_kwargs
    )
    # un-permute (s, p, k, t) -> row-major [J, K]
    out = np.concatenate(
        [
            res.results[c]["out"]
            .reshape(sup, P, K, T)
            .transpose(0, 1, 3, 2)
            .reshape(JC, K)
            for c in range(NCORES)
        ],
        axis=0,
    )
    return out, res


def kernel(input1: np.ndarray, input2: np.ndarray) -> np.ndarray:
    out, _ = _run(input1, input2)
    return out
